# revision 1
# baseline (speedup 1.0000x reference)
"""Trainium2 Bass kernel for nn_AttentionBlock (column-softmax causal attention).

Reference computation (B=4, S=4096, D=128, K=64, V=128):
    Q = x @ Wq.T + bq            [B,S,64]
    Km = x @ Wk.T + bk           [B,S,64]
    Vm = x @ Wv.T + bv           [B,S,128]
    s  = Q @ Km.T / 8            [B,S,S], causal mask j>q -> -1e9
    p  = softmax(s, axis=1)      (softmax over the QUERY axis -- column softmax)
    att = p @ Vm                 [B,S,128]
    out = concat(x, att, dim=2)  [B,S,256]

Key observation: with ST = s.T (layout [j, q]) the softmax denominator
l[j] = sum_q exp(ST[j, q]) is a free-dim reduction, so
att[q] = sum_j exp(ST[j,q]) * (Vm[j]/l[j]) -- a flash-style two-phase kernel
with NO max subtraction needed (scores are O(+-20), exp is safe in fp32).

Sharding (8 cores): core c -> batch b = c//2, j-tile parity p = c%2.
Each core computes l[j] and the PV partial sum for its 16 j-tiles
(j-tile J = 2*i + p), over all q. Host adds the two partials per batch.
All parity differences are data-driven (xkv row gather + additive mask
input), so one SPMD program serves all 8 cores.

Performance structure per core:
  - QK score matmuls run as f32r with row-PAIR packing: rows 2r / 2r+1 use
    the two 64-partition halves of the PE array concurrently (KT/QT are
    duplicated into both partition halves so tile_position auto-derives).
  - exp runs on ACT with fused per-partition accumulation (accum_out = l).
  - PV runs transposed: attT[v, q] = sum_j Vp[j,v] * E[j,q] with N=512
    moving operands (bf16), then PE-transposes back to [q, v] tiles.
"""

import numpy as np

B, S, D = 4, 4096, 128
KD, VD = 64, 128
P = 128
NCORES = 8
JT = 16           # local j-tiles per core
NT = S // P       # 32 global q/j tiles
CHUNK = 1024      # ACT exp chunk width (PSUM cols)

QK_F32R = True

ROW_W = [S - 2 * i * P for i in range(JT)]          # E row widths
EOFF = [0] * JT
for _i in range(1, JT):
    EOFF[_i] = EOFF[_i - 1] + ROW_W[_i - 1]
ECOLS = EOFF[-1] + ROW_W[-1]                        # 34816

_CACHE = {}


def _build_program():
    from contextlib import ExitStack

    from concourse import bacc, mybir
    from concourse import tile as tile_mod

    dt = mybir.dt
    f32, bf16 = dt.float32, dt.bfloat16
    Alu = mybir.AluOpType
    ActF = mybir.ActivationFunctionType

    nc = bacc.Bacc(
        "TRN2", target_bir_lowering=False, debug=False, num_devices=NCORES
    )

    # Operand tiles of f32r matmuls must be PRODUCED as float32r (the BIR
    # verifier requires the producing instruction to round); they are
    # written by DVE ops (which round) or DMA'd in as float32r directly.
    mmdt = dt.float32r if QK_F32R else f32

    # Host supplies x^T / xkv^T / W^T (pure layout prep) so the kernel
    # spends no PE/ACT/DVE time transposing, and the DMAs are contiguous.
    xt_d = nc.dram_tensor("xt", [P, S], mmdt, kind="ExternalInput").ap()
    xkvt_d = nc.dram_tensor("xkvt", [P, JT * P], mmdt, kind="ExternalInput").ap()
    # all small per-core inputs packed into one tensor (one DMA issue):
    # wqt[0:128] wkt[128:256] wvt[256:384] bq[384] bk[385] bv[386] mrow[387:643]
    small_d = nc.dram_tensor("small", [P, 643], mmdt, kind="ExternalInput").ap()
    att_d = nc.dram_tensor("att", [S, VD], f32, kind="ExternalOutput").ap()

    with tile_mod.TileContext(nc) as tc, ExitStack() as ctx:
        persist = ctx.enter_context(tc.tile_pool(name="persist", bufs=1))

        xT = persist.tile([P, S], mmdt)            # [d, q]
        xkvT = persist.tile([P, JT * P], mmdt)     # [d, local j]
        # QT/KT live in BOTH partition halves (rows 0-63 == rows 64-127) so
        # QK row pairs can use tile_position (0,0)/(64,0) concurrently.
        QT = persist.tile([P, S], mmdt)            # [k(dup), q]
        KTl = persist.tile([P, JT * P], mmdt)      # [k(dup), local j]
        V_sb = persist.tile([P, JT, VD], f32)      # [local j, v]
        Vp_sb = persist.tile([P, JT, VD], bf16)    # V / l
        E_all = persist.tile([P, ECOLS], bf16)     # exp(scores.T) rows
        l_all = persist.tile([P, JT], f32)
        linv = persist.tile([P, JT], f32)
        VT_sb = persist.tile([P, JT * P], f32)     # [v, local j]
        small_sb = persist.tile([P, 643], mmdt)
        ident = persist.tile([P, P], f32)
        WqT = small_sb[:, 0:128]
        WkT = small_sb[:, 128:256]
        WvT = small_sb[:, 256:384]
        bq_sb = small_sb[:, 384:385].bitcast(f32)
        bk_sb = small_sb[:, 385:386].bitcast(f32)
        bv_sb = small_sb[:, 386:387].bitcast(f32)
        mrow_sb = small_sb[:, 387:643].bitcast(f32)

        # ---- input DMAs: 5 issues total (SWDGE descriptor-gen is ~0.6us
        # of serial sequencer time PER dma_start), split across two
        # sequencers; high halves first (reverse pair order needs them).
        nc.gpsimd.dma_start(out=small_sb, in_=small_d)
        nc.gpsimd.dma_start(
            out=xkvT[:, 1024:2048], in_=xkvt_d[:, 1024:2048]
        )
        nc.gpsimd.dma_start(out=xkvT[:, 0:1024], in_=xkvt_d[:, 0:1024])
        nc.sync.dma_start(out=xT[:, 2048:4096], in_=xt_d[:, 2048:4096])
        nc.sync.dma_start(out=xT[:, 0:2048], in_=xt_d[:, 0:2048])

        # identity for PE transposes
        nc.gpsimd.memset(ident, 0.0)
        nc.gpsimd.affine_select(
            out=ident,
            in_=ident,
            compare_op=Alu.not_equal,
            fill=1.0,
            base=0,
            pattern=[[-1, P]],
            channel_multiplier=1,
        )

        # ---- phase A/B: row pairs in REVERSE order, each preceded only by
        # the KT/QT projection chunks it needs (so the PE FIFO never blocks
        # on input DMA for data a later pair needs), with the V path spread
        # through the (ACT-bound) pair phase.
        with ExitStack() as pha:
            prj = pha.enter_context(
                tc.tile_pool(name="prj_psum", bufs=2, space="PSUM")
            )
            rowp = pha.enter_context(
                tc.tile_pool(name="row_psum", bufs=3, space="PSUM")
            )
            lpp = pha.enter_context(tc.tile_pool(name="lparts", bufs=8))

            def emit_kt_chunk(c):
                ps = prj.tile([P, 512], f32, tag="prj", name=f"kt_{c}")
                nc.tensor.matmul(
                    ps,
                    lhsT=WkT,
                    rhs=xkvT[:, c * 512 : (c + 1) * 512],
                    start=True,
                    stop=True,
                )
                nc.vector.tensor_scalar(
                    out=KTl[:, c * 512 : (c + 1) * 512],
                    in0=ps,
                    scalar1=bk_sb,
                    scalar2=None,
                    op0=Alu.add,
                )

            def emit_qt_chunk(c):
                ps = prj.tile([P, 512], f32, tag="prj", name=f"qt_{c}")
                nc.tensor.matmul(
                    ps,
                    lhsT=WqT,
                    rhs=xT[:, c * 512 : (c + 1) * 512],
                    start=True,
                    stop=True,
                )
                nc.vector.tensor_scalar(
                    out=QT[:, c * 512 : (c + 1) * 512],
                    in0=ps,
                    scalar1=bq_sb,
                    scalar2=None,
                    op0=Alu.add,
                )

            def emit_v_group(g):
                # VT chunk g -> V tiles [j, v] for rows 4g..4g+3, then
                # V' = V/l (their l is complete once pair 2g is done).
                ps = prj.tile([P, 512], f32, tag="prj", name=f"vt_{g}")
                nc.tensor.matmul(
                    ps,
                    lhsT=WvT,
                    rhs=xkvT[:, g * 512 : (g + 1) * 512],
                    start=True,
                    stop=True,
                )
                nc.vector.tensor_scalar(
                    out=VT_sb[:, g * 512 : (g + 1) * 512],
                    in0=ps,
                    scalar1=bv_sb,
                    scalar2=None,
                    op0=Alu.add,
                )
                pst = prj.tile([P, 4, P], f32, tag="prj", name=f"vtp_{g}")
                for k in range(4):
                    i = g * 4 + k
                    nc.tensor.transpose(
                        pst[:, k, :], VT_sb[:, i * P : (i + 1) * P], ident
                    )
                nc.vector.tensor_copy(
                    V_sb[:, g * 4 : (g + 1) * 4, :].rearrange(
                        "p a b -> p (a b)"
                    ),
                    pst.rearrange("p a b -> p (a b)"),
                )
                for i in range(4 * g, 4 * g + 4):
                    nc.vector.reciprocal(
                        linv[:, i : i + 1], l_all[:, i : i + 1]
                    )
                    nc.vector.tensor_scalar(
                        out=Vp_sb[:, i, :],
                        in0=V_sb[:, i, :],
                        scalar1=linv[:, i : i + 1],
                        scalar2=None,
                        op0=Alu.mult,
                    )

            def emit_row_pair(r):
                # rows 2r (partition half 0) and 2r+1 (half 64), MMs
                # interleaved at 512-slice granularity so the PE overlaps
                # them in opposite array halves.
                state = {}
                for i in (2 * r, 2 * r + 1):
                    q0 = 256 * i
                    w = ROW_W[i]
                    chunks = [
                        (q0 + c * CHUNK, min(CHUNK, w - c * CHUNK))
                        for c in range((w + CHUNK - 1) // CHUNK)
                    ]
                    slices = []
                    for ci, (off, cw) in enumerate(chunks):
                        for s0 in range(0, cw, 512):
                            slices.append((ci, off, cw, s0, min(512, cw - s0)))
                    state[i] = {"chunks": chunks, "slices": slices, "ps": {}}

                def finish_chunk(i, ci, cw):
                    st = state[i]
                    ps = st["ps"][ci]
                    if ci == 0:
                        nc.vector.tensor_add(
                            ps[:, : 2 * P], ps[:, : 2 * P], mrow_sb
                        )
                    lp = lpp.tile([P, 1], f32, tag="lp", name=f"lp_{i}_{ci}")
                    ecol = EOFF[i] + ci * CHUNK
                    nc.scalar.activation(
                        out=E_all[:, ecol : ecol + cw],
                        in_=ps[:, :cw],
                        func=ActF.Exp,
                        accum_out=lp,
                    )
                    if ci == 0:
                        nc.vector.tensor_copy(l_all[:, i : i + 1], lp)
                    else:
                        nc.vector.tensor_add(
                            l_all[:, i : i + 1], l_all[:, i : i + 1], lp
                        )

                nslice = max(len(state[i]["slices"]) for i in state)
                for k in range(nslice):
                    for idx, i in enumerate((2 * r, 2 * r + 1)):
                        st = state[i]
                        if k >= len(st["slices"]):
                            continue
                        ci, off, cw, s0, sw = st["slices"][k]
                        if ci not in st["ps"]:
                            st["ps"][ci] = rowp.tile(
                                [P, CHUNK], f32, tag="st", name=f"st_{i}_{ci}"
                            )
                        base = KD * idx
                        nc.tensor.matmul(
                            st["ps"][ci][:, s0 : s0 + sw],
                            lhsT=KTl[base : base + KD, i * P : (i + 1) * P],
                            rhs=QT[base : base + KD, off + s0 : off + s0 + sw],
                            start=True,
                            stop=True,
                        )
                        if s0 + sw == cw:
                            finish_chunk(i, ci, cw)

            kt_done = set()
            for r in reversed(range(8)):
                if r // 2 not in kt_done:
                    kt_done.add(r // 2)
                    emit_kt_chunk(r // 2)
                emit_qt_chunk(r)
                emit_row_pair(r)
                if r % 2 == 0:
                    emit_v_group(r // 2)

        # ---- phase C: PV block ------------------------------------------
        with ExitStack() as phc:
            attp = phc.enter_context(
                tc.tile_pool(name="att_psum", bufs=4, space="PSUM")
            )
            tsbp = phc.enter_context(tc.tile_pool(name="attT_sb", bufs=2))
            sbo = phc.enter_context(tc.tile_pool(name="att_sb", bufs=2))

            def emit_pv_chunk(c):
                # attT[v, q] for q in [512c, 512c+512): rows i <= 2c full
                # coverage (N=512), row 2c+1 covers the second half (N=256).
                attT = attp.tile([P, 4, P], f32, tag="attT", name=f"attT_{c}")
                aflat = attT.rearrange("p a b -> p (a b)")
                for i in range(2 * c + 1):
                    ecol = EOFF[i] + 512 * c - 256 * i
                    nc.tensor.matmul(
                        aflat,
                        lhsT=Vp_sb[:, i, :],
                        rhs=E_all[:, ecol : ecol + 512],
                        start=(i == 0),
                        stop=False,
                    )
                i2 = 2 * c + 1
                nc.tensor.matmul(
                    aflat[:, 256:512],
                    lhsT=Vp_sb[:, i2, :],
                    rhs=E_all[:, EOFF[i2] : EOFF[i2] + 256],
                    start=False,
                    stop=True,
                )
                tsb = tsbp.tile([P, 4, P], f32, tag="tsb", name=f"tsb_{c}")
                nc.scalar.copy(tsb.rearrange("p a b -> p (a b)"), aflat)
                outq = attp.tile([P, 4, P], f32, tag="attT", name=f"outq_{c}")
                for k in range(4):
                    nc.tensor.transpose(outq[:, k, :], tsb[:, k, :], ident)
                osb = sbo.tile([P, 4, P], f32, tag="osb", name=f"osb_{c}")
                nc.vector.tensor_copy(
                    osb.rearrange("p a b -> p (a b)"),
                    outq.rearrange("p a b -> p (a b)"),
                )
                nc.sync.dma_start(
                    out=att_d[c * 512 : (c + 1) * 512, :].rearrange(
                        "(t p) v -> p t v", p=P
                    ),
                    in_=osb,
                )

            for c in reversed(range(8)):
                emit_pv_chunk(c)

    nc.compile()
    return nc


def _host_inputs(x, Wq, bq, Wk, bk, Wv, bv):
    """Per-core input maps (host does layout prep: transposes + gathers)."""
    x_full = np.ascontiguousarray(x, dtype=np.float32)
    Wq_s = np.asarray(Wq, np.float32) / 8.0
    wqt = np.ascontiguousarray(np.concatenate([Wq_s.T, Wq_s.T], axis=1))
    bq_s = np.tile((np.asarray(bq, np.float32) / 8.0).reshape(KD, 1), (2, 1))
    WkT_ = np.asarray(Wk, np.float32).T
    wkt = np.ascontiguousarray(np.concatenate([WkT_, WkT_], axis=1))
    bk_ = np.tile(np.asarray(bk, np.float32).reshape(KD, 1), (2, 1))
    wvt = np.ascontiguousarray(np.asarray(Wv, np.float32).T)
    bv_ = np.asarray(bv, np.float32).reshape(VD, 1)

    tri = np.where(
        np.arange(P)[None, :] >= np.arange(P)[:, None], 0.0, -1e9
    ).astype(np.float32)
    mrows = []
    for p in (0, 1):
        m = np.zeros((P, 2 * P), np.float32)
        if p == 0:
            m[:, :P] = tri
        else:
            m[:, :P] = -1e9
            m[:, P:] = tri
        mrows.append(m)

    smalls = [
        np.ascontiguousarray(
            np.concatenate([wqt, wkt, wvt, bq_s, bk_, bv_, mrows[p]], axis=1)
        )
        for p in (0, 1)
    ]
    in_maps = []
    xts = [np.ascontiguousarray(x_full[b].T) for b in range(B)]
    for c in range(NCORES):
        b, p = c // 2, c % 2
        xkvt = np.ascontiguousarray(
            x_full[b].reshape(NT, P, D)[p::2].reshape(JT * P, D).T
        )
        in_maps.append({"xt": xts[b], "xkvt": xkvt, "small": smalls[p]})
    return in_maps


def _get_program():
    if "nc" not in _CACHE:
        _CACHE["nc"] = _build_program()
    return _CACHE["nc"]


def run_on_device(in_maps, trace=False, trace_kwargs=None):
    from concourse import bass_utils

    nc = _get_program()
    return bass_utils.run_bass_kernel_spmd(
        nc,
        in_maps,
        core_ids=list(range(NCORES)),
        trace=trace,
        trace_kwargs=trace_kwargs or {},
    )


def kernel(x, Wq, bq, Wk, bk, Wv, bv):
    x = np.asarray(x, np.float32)
    in_maps = _host_inputs(x, Wq, bq, Wk, bk, Wv, bv)
    res = run_on_device(in_maps)
    att = np.empty((B, S, VD), np.float32)
    for b in range(B):
        att[b] = res.results[2 * b]["att"] + res.results[2 * b + 1]["att"]
    return np.concatenate([x, att], axis=2)



# revision 3
# speedup vs baseline: 1.1936x; 1.1936x over previous
"""Trainium2 Bass kernel for nn_AttentionBlock (column-softmax causal attention).

Reference computation (B=4, S=4096, D=128, K=64, V=128):
    Q = x @ Wq.T + bq            [B,S,64]
    Km = x @ Wk.T + bk           [B,S,64]
    Vm = x @ Wv.T + bv           [B,S,128]
    s  = Q @ Km.T / 8            [B,S,S], causal mask j>q -> -1e9
    p  = softmax(s, axis=1)      (softmax over the QUERY axis -- column softmax)
    att = p @ Vm                 [B,S,128]
    out = concat(x, att, dim=2)  [B,S,256]

With ST = s.T (layout [j, q]) the softmax denominator l[j] = sum_q exp(ST[j,q])
is a free-dim reduction (ACT accum), so att^T[v,q] = sum_j (Vm[j,v]/l[j]) *
exp(ST[j,q]) -- no max subtraction needed (scores are O(+-5), fp32 exp safe).

Sharding (8 cores): core c -> batch b = c//2, j-tile parity p = c%2.
Each core handles its 16 j-tiles (J = 2*i + p) over all q; the host adds the
two partial att^T per batch and transposes. All parity differences are
data-driven (xkv row gather + additive mask input): one SPMD program.

Performance structure per core (vs the 114us baseline):
  - all PE operands are bf16 (halves input DMA, full PE rate);
  - rows processed FORWARD with PV chunks interleaved (pv chunk c right
    after qk row 2c+2), so the PE stream is dense start-to-finish: it
    holds the top p-state (2.4 GHz needs ~3us continuous busy) and the
    old 2.1us phase gap + PV-phase PSUM gaps disappear;
  - V is produced directly in [j, v] layout (lhsT=xkv tile, rhs=Wv^T):
    no PE transposes anywhere; bv is added via a broadcast row tile;
  - output stays transposed ([v, q] per 512-chunk, DMA'd as computed);
    the host adds partials and transposes -- removes 32 PE transposes,
    8 ACT copies, 8 DVE copies and the serial output tail;
  - exp runs on ACT with fused per-partition accumulation (accum_out=l),
    table preloaded at t=0 via a dummy exp.
"""

import numpy as np

B, S, D = 4, 4096, 128
KD, VD = 64, 128
P = 128
NCORES = 8
JT = 16           # local j-tiles per core
NT = S // P       # 32 global j tiles
CHUNK = 1024      # ACT exp chunk width (PSUM cols)

ROW_W = [S - 2 * i * P for i in range(JT)]          # E row widths
EOFF = [0] * JT
for _i in range(1, JT):
    EOFF[_i] = EOFF[_i - 1] + ROW_W[_i - 1]
ECOLS = EOFF[-1] + ROW_W[-1]                        # 34816

_CACHE = {}


def _build_program():
    from contextlib import ExitStack

    from concourse import bacc, mybir
    from concourse import tile as tile_mod

    dt = mybir.dt
    f32, bf16 = dt.float32, dt.bfloat16
    Alu = mybir.AluOpType
    ActF = mybir.ActivationFunctionType

    nc = bacc.Bacc(
        "TRN2", target_bir_lowering=False, debug=False, num_devices=NCORES
    )

    # Host supplies x^T / xkv^T / W^T in bf16 (pure layout prep; halves DMA)
    xt_d = nc.dram_tensor("xt", [P, S], bf16, kind="ExternalInput").ap()
    xkvt_d = nc.dram_tensor("xkvt", [P, JT * P], bf16, kind="ExternalInput").ap()
    # wq^T/8 [0:64] | wk^T [64:128] | wv^T [128:256]
    smallw_d = nc.dram_tensor("smallw", [P, 256], bf16, kind="ExternalInput").ap()
    # bq/8 [0] | bk [1] | bv broadcast rows [2:130] | mask rows [130:386]
    smallf_d = nc.dram_tensor("smallf", [P, 386], f32, kind="ExternalInput").ap()
    att_d = nc.dram_tensor("att", [P, S], f32, kind="ExternalOutput").ap()

    with tile_mod.TileContext(nc) as tc, ExitStack() as ctx:
        persist = ctx.enter_context(tc.tile_pool(name="persist", bufs=1))

        xT = persist.tile([P, S], bf16)            # [d, q]
        xkvT = persist.tile([P, JT * P], bf16)     # [d, local j]
        QT = persist.tile([KD, S], bf16)           # [k, q]
        KTl = persist.tile([KD, JT * P], bf16)     # [k, local j]
        V_sb = persist.tile([P, JT, VD], f32)      # [local j, v]
        Vp_sb = persist.tile([P, JT, VD], bf16)    # V / l
        E_all = persist.tile([P, ECOLS], bf16)     # exp(scores.T) rows
        l_all = persist.tile([P, JT], f32)
        linv = persist.tile([P, JT], f32)
        smallw_sb = persist.tile([P, 256], bf16)
        smallf_sb = persist.tile([P, 386], f32)
        wup = persist.tile([P, 2], f32)            # act-table warmup scratch

        WqT = smallw_sb[:, 0:KD]
        WkT = smallw_sb[:, KD : 2 * KD]
        WvT = smallw_sb[:, 2 * KD : 2 * KD + VD]
        bq_sb = smallf_sb[0:KD, 0:1]
        bk_sb = smallf_sb[0:KD, 1:2]
        bvB = smallf_sb[:, 2 : 2 + VD]
        mrow = smallf_sb[:, 2 + VD : 2 + VD + 2 * P]

        # ---- input DMAs (SWDGE descriptor-gen is ~0.6us serial per
        # dma_start on the issuing queue -> spread across 3 queues)
        nc.gpsimd.dma_start(out=smallw_sb, in_=smallw_d)
        nc.gpsimd.dma_start(out=xkvT[:, 0:1024], in_=xkvt_d[:, 0:1024])
        nc.gpsimd.dma_start(out=xkvT[:, 1024:2048], in_=xkvt_d[:, 1024:2048])
        nc.scalar.dma_start(out=smallf_sb, in_=smallf_d)
        for q4 in range(4):
            nc.sync.dma_start(
                out=xT[:, q4 * 1024 : (q4 + 1) * 1024],
                in_=xt_d[:, q4 * 1024 : (q4 + 1) * 1024],
            )

        # preload the ACT exp table at t~0 so the first real exp doesn't
        # pay the 1.3us table load
        nc.gpsimd.memset(wup[:, 0:1], 0.0)
        nc.scalar.activation(out=wup[:, 1:2], in_=wup[:, 0:1], func=ActF.Exp)

        prj = ctx.enter_context(tc.tile_pool(name="prj", bufs=2, space="PSUM"))
        rowp = ctx.enter_context(tc.tile_pool(name="rowp", bufs=3, space="PSUM"))
        lpp = ctx.enter_context(tc.tile_pool(name="lpp", bufs=8))
        sbo = ctx.enter_context(tc.tile_pool(name="sbo", bufs=2))

        def emit_k_chunk(c4):
            ps = prj.tile([KD, 512], f32, tag="prj", name=f"kt_{c4}")
            nc.tensor.matmul(
                ps,
                lhsT=WkT,
                rhs=xkvT[:, c4 * 512 : (c4 + 1) * 512],
                start=True,
                stop=True,
            )
            nc.vector.tensor_scalar(
                out=KTl[:, c4 * 512 : (c4 + 1) * 512],
                in0=ps,
                scalar1=bk_sb,
                scalar2=None,
                op0=Alu.add,
            )

        def emit_q_chunk(c):
            ps = prj.tile([KD, 512], f32, tag="prj", name=f"qt_{c}")
            nc.tensor.matmul(
                ps,
                lhsT=WqT,
                rhs=xT[:, c * 512 : (c + 1) * 512],
                start=True,
                stop=True,
            )
            nc.vector.tensor_scalar(
                out=QT[:, c * 512 : (c + 1) * 512],
                in0=ps,
                scalar1=bq_sb,
                scalar2=None,
                op0=Alu.add,
            )

        def emit_v_tile(t):
            # V tile directly in [j, v] layout; bias added via broadcast rows
            ps = prj.tile([P, VD], f32, tag="prj", name=f"v_{t}")
            nc.tensor.matmul(
                ps,
                lhsT=xkvT[:, t * P : (t + 1) * P],
                rhs=WvT,
                start=True,
                stop=True,
            )
            nc.vector.tensor_add(V_sb[:, t, :], ps, bvB)

        def emit_qk_row(i):
            q0 = 256 * i
            w = ROW_W[i]
            nch = (w + CHUNK - 1) // CHUNK
            for ci in range(nch):
                coff = ci * CHUNK
                cw = min(CHUNK, w - coff)
                ps = rowp.tile([P, CHUNK], f32, tag="st", name=f"st_{i}_{ci}")
                for s0 in range(0, cw, 512):
                    sw = min(512, cw - s0)
                    nc.tensor.matmul(
                        ps[:, s0 : s0 + sw],
                        lhsT=KTl[:, i * P : (i + 1) * P],
                        rhs=QT[:, q0 + coff + s0 : q0 + coff + s0 + sw],
                        start=True,
                        stop=True,
                    )
                if ci == 0:
                    nc.vector.tensor_add(
                        ps[:, : 2 * P], ps[:, : 2 * P], mrow
                    )
                lp = lpp.tile([P, 1], f32, tag="lp", name=f"lp_{i}_{ci}")
                ecol = EOFF[i] + coff
                nc.scalar.activation(
                    out=E_all[:, ecol : ecol + cw],
                    in_=ps[:, :cw],
                    func=ActF.Exp,
                    accum_out=lp,
                )
                if ci == 0:
                    nc.vector.tensor_copy(l_all[:, i : i + 1], lp)
                else:
                    nc.vector.tensor_add(
                        l_all[:, i : i + 1], l_all[:, i : i + 1], lp
                    )
            nc.vector.reciprocal(linv[:, i : i + 1], l_all[:, i : i + 1])
            nc.vector.tensor_scalar(
                out=Vp_sb[:, i, :],
                in0=V_sb[:, i, :],
                scalar1=linv[:, i : i + 1],
                scalar2=None,
                op0=Alu.mult,
            )

        def emit_pv_chunk(c):
            # att^T[v, q] for q in [512c, 512c+512): rows i <= 2c full
            # coverage (N=512), row 2c+1 covers the second half (N=256).
            # shares the "prj" tag's 2 slots: PSUM total = 2 (prj) + 6 (st)
            ps = prj.tile([P, 512], f32, tag="prj", name=f"pv_{c}")
            for i2 in range(2 * c + 1):
                ecol = EOFF[i2] + 512 * c - 256 * i2
                nc.tensor.matmul(
                    ps,
                    lhsT=Vp_sb[:, i2, :],
                    rhs=E_all[:, ecol : ecol + 512],
                    start=(i2 == 0),
                    stop=False,
                )
            i2 = 2 * c + 1
            nc.tensor.matmul(
                ps[:, 256:512],
                lhsT=Vp_sb[:, i2, :],
                rhs=E_all[:, EOFF[i2] : EOFF[i2] + 256],
                start=False,
                stop=True,
            )
            osb = sbo.tile([P, 512], f32, tag="osb", name=f"osb_{c}")
            nc.vector.tensor_copy(osb, ps)
            nc.sync.dma_start(
                out=att_d[:, c * 512 : (c + 1) * 512], in_=osb
            )

        # ---- preamble: K chunk 0 (rows 0-3), Q chunks chasing the xT DMA,
        # first two V tiles. Remaining K chunks / V tiles are sprinkled
        # into the row phase right before they're needed.
        emit_k_chunk(0)
        for c in range(8):
            emit_q_chunk(c)
        emit_v_tile(0)
        emit_v_tile(1)

        # ---- main pipeline: forward rows, pv chunk c after row 2c+2
        for i in range(JT):
            if i >= 2:
                emit_v_tile(i)
            if i in (4, 8, 12):
                emit_k_chunk(i // 4)
            emit_qk_row(i)
            if i >= 2 and i % 2 == 0:
                emit_pv_chunk(i // 2 - 1)
        emit_pv_chunk(6)
        emit_pv_chunk(7)

    nc.compile()
    return nc


def _host_inputs(x, Wq, bq, Wk, bk, Wv, bv):
    """Per-core input maps (host does layout prep: transposes + gathers)."""
    import ml_dtypes

    bf16 = ml_dtypes.bfloat16
    x_full = np.ascontiguousarray(x, dtype=np.float32)
    xb = x_full.astype(bf16)
    wq8 = (np.asarray(Wq, np.float32).T / 8.0).astype(bf16)   # [128, 64]
    wk_ = np.asarray(Wk, np.float32).T.astype(bf16)           # [128, 64]
    wv_ = np.asarray(Wv, np.float32).T.astype(bf16)           # [128, 128]
    smallw = np.ascontiguousarray(np.concatenate([wq8, wk_, wv_], axis=1))

    bq8 = np.zeros((P, 1), np.float32)
    bq8[:KD, 0] = np.asarray(bq, np.float32) / 8.0
    bk_c = np.zeros((P, 1), np.float32)
    bk_c[:KD, 0] = np.asarray(bk, np.float32)
    bvB = np.tile(np.asarray(bv, np.float32)[None, :], (P, 1))

    tri = np.where(
        np.arange(P)[None, :] >= np.arange(P)[:, None], 0.0, -1e9
    ).astype(np.float32)
    smallfs = []
    for p in (0, 1):
        m = np.zeros((P, 2 * P), np.float32)
        if p == 0:
            m[:, :P] = tri
        else:
            m[:, :P] = -1e9
            m[:, P:] = tri
        smallfs.append(
            np.ascontiguousarray(
                np.concatenate([bq8, bk_c, bvB, m], axis=1)
            )
        )

    in_maps = []
    xts = [np.ascontiguousarray(xb[b].T) for b in range(B)]
    for c in range(NCORES):
        b, p = c // 2, c % 2
        xkvt = np.ascontiguousarray(
            xb[b].reshape(NT, P, D)[p::2].reshape(JT * P, D).T
        )
        in_maps.append(
            {"xt": xts[b], "xkvt": xkvt, "smallw": smallw, "smallf": smallfs[p]}
        )
    return in_maps


def _get_program():
    if "nc" not in _CACHE:
        _CACHE["nc"] = _build_program()
    return _CACHE["nc"]


def run_on_device(in_maps, trace=False, trace_kwargs=None):
    from concourse import bass_utils

    nc = _get_program()
    return bass_utils.run_bass_kernel_spmd(
        nc,
        in_maps,
        core_ids=list(range(NCORES)),
        trace=trace,
        trace_kwargs=trace_kwargs or {},
    )


def kernel(x, Wq, bq, Wk, bk, Wv, bv):
    x = np.asarray(x, np.float32)
    in_maps = _host_inputs(x, Wq, bq, Wk, bk, Wv, bv)
    res = run_on_device(in_maps)
    att = np.empty((B, S, VD), np.float32)
    for b in range(B):
        attT = res.results[2 * b]["att"] + res.results[2 * b + 1]["att"]
        att[b] = attT.T
    return np.concatenate([x, att], axis=2)


# revision 4
# speedup vs baseline: 1.4323x; 1.2000x over previous
"""Trainium2 Bass kernel for nn_AttentionBlock (column-softmax causal attention).

Reference computation (B=4, S=4096, D=128, K=64, V=128):
    Q = x @ Wq.T + bq            [B,S,64]
    Km = x @ Wk.T + bk           [B,S,64]
    Vm = x @ Wv.T + bv           [B,S,128]
    s  = Q @ Km.T / 8            [B,S,S], causal mask j>q -> -1e9
    p  = softmax(s, axis=1)      (softmax over the QUERY axis -- column softmax)
    att = p @ Vm                 [B,S,128]
    out = concat(x, att, dim=2)  [B,S,256]

Key algebra: s[q,j]/1 = x_q^T A x_j + u.x_q + (v.x_j + c) with
A = Wq^T Wk / 8, u = Wq^T bk / 8.  The per-j terms (v.x_j + c) are
constant along the softmax axis (q) and CANCEL in p = e/sum_q(e), so
they are dropped entirely.  Per j-tile the kernel computes
Bi[d,j] = sum_e A[d,e] x_j[e] + u[d] (one N=128 matmul + the psum->sbuf
copy that adds u), then streams scores^T rows as 128-contraction
matmuls  ST[j,q] = sum_d Bi[d,j] xT[d,q]  directly off the raw x^T --
no Q/K projections at all.

With ST in [j, q] layout the softmax denominator l[j] = sum_q exp(ST)
is a free-dim reduction (ACT accum_out); att^T[v,q] =
sum_j (Vm[j,v]/l[j]) * exp(ST[j,q]) -- no max subtraction needed
(scores are O(+-5), fp32 exp is safe).

Sharding (8 cores): core c -> batch b = c//2, j-tile parity p = c%2.
Each core handles its 16 j-tiles (J = 2*i + p) over all q; the host
adds the two partial att^T per batch and transposes. All parity
differences are data-driven (xkv row gather + additive mask input):
one SPMD program.

Performance structure per core (vs the 114us baseline):
  - all PE operands bf16 (halves input DMA, full PE rate), all
    matmuls 128-contraction;
  - rows processed FORWARD with PV chunks interleaved (pv chunk c
    right after qk row 2c+2) so the PE stream is dense start-to-finish
    and holds the top p-state (the PE clock ramps over tens of us --
    a few warmup matmuls start the ramp during the input DMA);
  - V is produced directly in [j, v] layout (lhsT=xkv tile, rhs=Wv^T):
    no PE transposes anywhere; bv added via a broadcast row tile;
  - output stays transposed ([v, q] per 512-chunk, DMA'd as computed);
    the host adds partials and transposes;
  - exp runs on ACT in 1536-wide chunks with fused per-partition
    accumulation (accum_out), table preloaded at t~0 via a dummy exp.
"""

import numpy as np

B, S, D = 4, 4096, 128
KD, VD = 64, 128
P = 128
NCORES = 8
JT = 16           # local j-tiles per core
NT = S // P       # 32 global j tiles
CHUNK = 1536      # ACT exp chunk width (3 PSUM banks)
NWARM = 6         # PE p-state warmup matmuls during input DMA

ROW_W = [S - 2 * i * P for i in range(JT)]          # E row widths
EOFF = [0] * JT
for _i in range(1, JT):
    EOFF[_i] = EOFF[_i - 1] + ROW_W[_i - 1]
ECOLS = EOFF[-1] + ROW_W[-1]                        # 34816

_CACHE = {}


def _build_program():
    from contextlib import ExitStack

    from concourse import bacc, mybir
    from concourse import tile as tile_mod

    dt = mybir.dt
    f32, bf16 = dt.float32, dt.bfloat16
    Alu = mybir.AluOpType
    ActF = mybir.ActivationFunctionType

    nc = bacc.Bacc(
        "TRN2", target_bir_lowering=False, debug=False, num_devices=NCORES
    )

    # Host supplies x^T / xkv^T / A^T / Wv^T in bf16 (pure layout prep)
    xt_d = nc.dram_tensor("xt", [P, S], bf16, kind="ExternalInput").ap()
    xkvt_d = nc.dram_tensor("xkvt", [P, JT * P], bf16, kind="ExternalInput").ap()
    # At [0:128] | wv^T [128:256]
    smallw_d = nc.dram_tensor("smallw", [P, 256], bf16, kind="ExternalInput").ap()
    # u [0] | bv broadcast rows [1:129] | mask rows [129:385]
    smallf_d = nc.dram_tensor("smallf", [P, 385], f32, kind="ExternalInput").ap()
    att_d = nc.dram_tensor("att", [P, S], f32, kind="ExternalOutput").ap()

    with tile_mod.TileContext(nc) as tc, ExitStack() as ctx:
        persist = ctx.enter_context(tc.tile_pool(name="persist", bufs=1))

        xT = persist.tile([P, S], bf16)            # [d, q]
        xkvT = persist.tile([P, JT * P], bf16)     # [d, local j]
        V_sb = persist.tile([P, JT, VD], f32)      # [local j, v]
        Vp_sb = persist.tile([P, JT, VD], bf16)    # V / l
        E_all = persist.tile([P, ECOLS], bf16)     # exp(scores.T) rows
        l_all = persist.tile([P, JT], f32)
        linv = persist.tile([P, JT], f32)
        smallw_sb = persist.tile([P, 256], bf16)
        smallf_sb = persist.tile([P, 385], f32)
        wup = persist.tile([P, 2], f32)            # act-table warmup scratch
        warm = persist.tile([P, 256], bf16)        # PE warmup operand

        At_sb = smallw_sb[:, 0:P]
        WvT = smallw_sb[:, P : P + VD]
        u_col = smallf_sb[:, 0:1]
        bvB = smallf_sb[:, 1 : 1 + VD]
        mrow = smallf_sb[:, 1 + VD : 1 + VD + 2 * P]

        # ---- engine warmups (before any data dependency)
        nc.vector.memset(wup[:, 0:1], 0.0)
        nc.scalar.activation(out=wup[:, 1:2], in_=wup[:, 0:1], func=ActF.Exp)
        nc.vector.memset(warm, 0.0)

        # ---- input DMAs (SWDGE descriptor-gen is ~0.6us serial per
        # dma_start on the issuing queue -> spread across 3 queues;
        # first-needed pieces first)
        nc.sync.dma_start(out=smallw_sb, in_=smallw_d)
        nc.sync.dma_start(out=xkvT[:, 0:256], in_=xkvt_d[:, 0:256])
        nc.sync.dma_start(out=xkvT[:, 256:1024], in_=xkvt_d[:, 256:1024])
        nc.sync.dma_start(out=xkvT[:, 1024:2048], in_=xkvt_d[:, 1024:2048])
        nc.scalar.dma_start(out=smallf_sb, in_=smallf_d)
        for q4 in range(4):
            nc.gpsimd.dma_start(
                out=xT[:, q4 * 1024 : (q4 + 1) * 1024],
                in_=xt_d[:, q4 * 1024 : (q4 + 1) * 1024],
            )

        prj = ctx.enter_context(tc.tile_pool(name="prj", bufs=2, space="PSUM"))
        rowp = ctx.enter_context(tc.tile_pool(name="rowp", bufs=2, space="PSUM"))
        lpp = ctx.enter_context(tc.tile_pool(name="lpp", bufs=8))
        bip = ctx.enter_context(tc.tile_pool(name="bip", bufs=3))
        sbo = ctx.enter_context(tc.tile_pool(name="sbo", bufs=2))

        # PE p-state warmup: matmuls on zeroed operands, no consumers
        for wi in range(NWARM):
            wp = prj.tile([P, 256], f32, tag="prj", name=f"warm_{wi}")
            nc.tensor.matmul(
                wp, lhsT=warm[:, 0:P], rhs=warm, start=True, stop=True
            )

        BIS = {}

        def emit_bi(i):
            # Bi[d, j] = sum_e A[d,e] xkv[e, j-tile i] + u[d]  (bf16)
            ps = prj.tile([P, P], f32, tag="prj", name=f"bi_{i}")
            nc.tensor.matmul(
                ps,
                lhsT=At_sb,
                rhs=xkvT[:, i * P : (i + 1) * P],
                start=True,
                stop=True,
            )
            bi = bip.tile([P, P], bf16, tag="bi", name=f"bis_{i}")
            nc.vector.tensor_scalar(
                out=bi, in0=ps, scalar1=u_col, scalar2=None, op0=Alu.add
            )
            BIS[i] = bi

        def emit_v_tile(t):
            # V tile directly in [j, v] layout; bias added via broadcast rows
            ps = prj.tile([P, VD], f32, tag="prj", name=f"v_{t}")
            nc.tensor.matmul(
                ps,
                lhsT=xkvT[:, t * P : (t + 1) * P],
                rhs=WvT,
                start=True,
                stop=True,
            )
            nc.vector.tensor_add(V_sb[:, t, :], ps, bvB)

        def emit_qk_row(i):
            q0 = 256 * i
            w = ROW_W[i]
            nch = (w + CHUNK - 1) // CHUNK
            for ci in range(nch):
                coff = ci * CHUNK
                cw = min(CHUNK, w - coff)
                ps = rowp.tile([P, CHUNK], f32, tag="st", name=f"st_{i}_{ci}")
                for s0 in range(0, cw, 512):
                    sw = min(512, cw - s0)
                    nc.tensor.matmul(
                        ps[:, s0 : s0 + sw],
                        lhsT=BIS[i],
                        rhs=xT[:, q0 + coff + s0 : q0 + coff + s0 + sw],
                        start=True,
                        stop=True,
                    )
                if ci == 0:
                    nc.vector.tensor_add(
                        ps[:, : 2 * P], ps[:, : 2 * P], mrow
                    )
                    # compute next row's Bi while this row streams
                    if i + 1 < JT:
                        emit_bi(i + 1)
                lp = lpp.tile([P, 1], f32, tag="lp", name=f"lp_{i}_{ci}")
                ecol = EOFF[i] + coff
                nc.scalar.activation(
                    out=E_all[:, ecol : ecol + cw],
                    in_=ps[:, :cw],
                    func=ActF.Exp,
                    accum_out=lp,
                )
                if ci == 0:
                    nc.vector.tensor_copy(l_all[:, i : i + 1], lp)
                else:
                    nc.vector.tensor_add(
                        l_all[:, i : i + 1], l_all[:, i : i + 1], lp
                    )
            nc.vector.reciprocal(linv[:, i : i + 1], l_all[:, i : i + 1])
            nc.vector.tensor_scalar(
                out=Vp_sb[:, i, :],
                in0=V_sb[:, i, :],
                scalar1=linv[:, i : i + 1],
                scalar2=None,
                op0=Alu.mult,
            )

        def emit_pv_chunk(c):
            # att^T[v, q] for q in [512c, 512c+512): rows i <= 2c full
            # coverage (N=512), row 2c+1 covers the second half (N=256).
            ps = prj.tile([P, 512], f32, tag="prj", name=f"pv_{c}")
            for i2 in range(2 * c + 1):
                ecol = EOFF[i2] + 512 * c - 256 * i2
                nc.tensor.matmul(
                    ps,
                    lhsT=Vp_sb[:, i2, :],
                    rhs=E_all[:, ecol : ecol + 512],
                    start=(i2 == 0),
                    stop=False,
                )
            i2 = 2 * c + 1
            nc.tensor.matmul(
                ps[:, 256:512],
                lhsT=Vp_sb[:, i2, :],
                rhs=E_all[:, EOFF[i2] : EOFF[i2] + 256],
                start=False,
                stop=True,
            )
            osb = sbo.tile([P, 512], f32, tag="osb", name=f"osb_{c}")
            nc.vector.tensor_copy(osb, ps)
            nc.sync.dma_start(
                out=att_d[:, c * 512 : (c + 1) * 512], in_=osb
            )

        # ---- preamble, then main pipeline: forward rows,
        # pv chunk c right after row 2c+2
        emit_bi(0)
        emit_v_tile(0)
        emit_v_tile(1)
        for i in range(JT):
            if i >= 2:
                emit_v_tile(i)
            emit_qk_row(i)
            if i >= 2 and i % 2 == 0:
                emit_pv_chunk(i // 2 - 1)
        emit_pv_chunk(6)
        emit_pv_chunk(7)

    nc.compile()
    return nc


def _host_inputs(x, Wq, bq, Wk, bk, Wv, bv):
    """Per-core input maps (host does layout prep: transposes + gathers)."""
    import ml_dtypes

    bf16 = ml_dtypes.bfloat16
    x_full = np.ascontiguousarray(x, dtype=np.float32)
    xb = x_full.astype(bf16)
    Wq32 = np.asarray(Wq, np.float32)
    Wk32 = np.asarray(Wk, np.float32)
    At = ((Wk32.T / 8.0) @ Wq32).astype(bf16)                 # [128, 128]
    wv_ = np.asarray(Wv, np.float32).T.astype(bf16)           # [128, 128]
    smallw = np.ascontiguousarray(np.concatenate([At, wv_], axis=1))

    u_c = (Wq32.T @ np.asarray(bk, np.float32) / 8.0).reshape(P, 1)
    bvB = np.tile(np.asarray(bv, np.float32)[None, :], (P, 1))

    tri = np.where(
        np.arange(P)[None, :] >= np.arange(P)[:, None], 0.0, -1e9
    ).astype(np.float32)
    smallfs = []
    for p in (0, 1):
        m = np.zeros((P, 2 * P), np.float32)
        if p == 0:
            m[:, :P] = tri
        else:
            m[:, :P] = -1e9
            m[:, P:] = tri
        smallfs.append(
            np.ascontiguousarray(
                np.concatenate([u_c, bvB, m], axis=1).astype(np.float32)
            )
        )

    in_maps = []
    xts = [np.ascontiguousarray(xb[b].T) for b in range(B)]
    for c in range(NCORES):
        b, p = c // 2, c % 2
        xkvt = np.ascontiguousarray(
            xb[b].reshape(NT, P, D)[p::2].reshape(JT * P, D).T
        )
        in_maps.append(
            {"xt": xts[b], "xkvt": xkvt, "smallw": smallw, "smallf": smallfs[p]}
        )
    return in_maps


def _get_program():
    if "nc" not in _CACHE:
        _CACHE["nc"] = _build_program()
    return _CACHE["nc"]


def run_on_device(in_maps, trace=False, trace_kwargs=None):
    from concourse import bass_utils

    nc = _get_program()
    return bass_utils.run_bass_kernel_spmd(
        nc,
        in_maps,
        core_ids=list(range(NCORES)),
        trace=trace,
        trace_kwargs=trace_kwargs or {},
    )


def kernel(x, Wq, bq, Wk, bk, Wv, bv):
    x = np.asarray(x, np.float32)
    in_maps = _host_inputs(x, Wq, bq, Wk, bk, Wv, bv)
    res = run_on_device(in_maps)
    att = np.empty((B, S, VD), np.float32)
    for b in range(B):
        attT = res.results[2 * b]["att"] + res.results[2 * b + 1]["att"]
        att[b] = attT.T
    return np.concatenate([x, att], axis=2)
